# revision 1
# baseline (speedup 1.0000x reference)
"""Bass/Trainium2 kernel for a 2-layer LSTM (B=512, T=2048, I=3, H=64).

Returns the final hidden state of layer 2, shape (512, 64) fp32.

Strategy (data-parallel over batch, 8 cores x 64 batch each):
  - All recurrent state lives in SBUF for the whole T=2048 recurrence.
  - State convention: ht = 2*h stored transposed (H on partitions, batch on
    free dim) in one (128, BL) fp16 tile: rows 0-63 = ht1 (layer1),
    rows 64-127 = ht2 (layer2).  Weights that multiply ht carry a 0.5.
  - sigmoid(z) = (tanh(z/2)+1)/2: the 0.5 is baked into the i/f/o gate
    weights, so ONE tanh ACTIVATE covers all four gates of a layer.
  - Cell state kept as c2x = 2*c in fp32; tanh(c) = tanh(0.5*c2x) via the
    ACT scale field.
  - x and the biases enter through a K=4 matmul (rows: x0,x1,x2,ones) from
    a host-pretransposed (4, T*BL) fp16 tensor, DMA'd in chunks.
  - The two layers run staggered by one timestep as two interleaved
    dependency chains.

Gate algebra per layer per step (i,f,g,o; ti=tanh(zi/2) etc, tg=tanh(zg)):
  u   = (ti + 1) * tg          # = 2*i*g            scalar_tensor_tensor
  w   = (0.5*c2x) * tf         # = tf*c             scalar_tensor_tensor
  s   = u + w                                        tensor_tensor
  c2x = 0.5*c2x + s            # = 2(f*c + i*g)     scalar_tensor_tensor
  tc  = tanh(0.5*c2x)                                ACT
  ht  = (to + 1) * tc          # = 2*o*tanh(c)      scalar_tensor_tensor
"""

import numpy as np

B, T, I, H = 512, 2048, 3, 64
NCORES = 8
BL = B // NCORES  # 64 batch per core
CH = 64  # timesteps per x-chunk DMA

_CACHE = {}


def _prep_weights(W_ih0, W_hh0, b_ih0, b_hh0, W_ih1, W_hh1, b_ih1, b_hh1):
    """Pack host-side lhsT weight arrays (fp16).

    Column order within each 256-col block: [i(64) | f(64) | g(64) | o(64)],
    i.e. if-block = cols 0..127, go-block = cols 128..255.
    """
    sg = np.concatenate(
        [np.full(H, 0.5), np.full(H, 0.5), np.full(H, 1.0), np.full(H, 0.5)]
    ).astype(np.float32)  # tanh-arg scale per gate row (i,f,g,o)

    b0 = (b_ih0 + b_hh0) * sg
    b1 = (b_ih1 + b_hh1) * sg
    Wx0 = W_ih0 * sg[:, None]  # acts on true x
    Wh0 = W_hh0 * sg[:, None] * 0.5  # acts on ht1 = 2*h1
    Wi1 = W_ih1 * sg[:, None] * 0.5  # acts on ht1
    Wh1 = W_hh1 * sg[:, None] * 0.5  # acts on ht2

    # Gate column order: layer 1 uses [f,i,o,g] so its elementwise algebra is
    # partition-aligned in rows 0-63; layer 2 uses [i,f,g,o] (aligned in rows
    # 64-127).  See cell_update.
    p1 = np.r_[H : 2 * H, 0:H, 3 * H : 4 * H, 2 * H : 3 * H]

    # w13: (68, 512).  cols 0-255: layer-1 lhsT (state rows 0-63, x rows
    # 64-66, bias row 67).  cols 256-511: layer-2 x-block lhsT (rows 64-66
    # zero, row 67 = layer-2 bias) -- rides the same K=4 rhs.
    w13 = np.zeros((68, 512), np.float32)
    w13[0:64, 0:256] = Wh0.T[:, p1]
    w13[64:67, 0:256] = Wx0.T[:, p1]
    w13[67, 0:256] = b0[p1]
    w13[67, 256:512] = b1
    # w2: (128, 256) layer-2 state lhsT: rows 0-63 act on ht1, 64-127 on ht2.
    w2 = np.concatenate([Wi1.T, Wh1.T], axis=0)
    return w13.astype(np.float16), np.ascontiguousarray(w2).astype(np.float16)


def build_program(t_steps=T, bl=BL):
    """Build the Bass program (one core's SPMD program)."""
    import concourse.bass as bass
    import concourse.tile as tile
    from concourse import bacc, mybir

    f32 = mybir.dt.float32
    f16 = mybir.dt.float16
    Tanh = mybir.ActivationFunctionType.Tanh
    ADD = mybir.AluOpType.add
    MULT = mybir.AluOpType.mult

    nc = bacc.Bacc("TRN2", target_bir_lowering=False, debug=False)

    xt_d = nc.dram_tensor("xt", [4, t_steps * bl], f16, kind="ExternalInput")
    w13_d = nc.dram_tensor("w13", [68, 512], f16, kind="ExternalInput")
    w2_d = nc.dram_tensor("w2", [128, 256], f16, kind="ExternalInput")
    out_d = nc.dram_tensor("out", [64, bl], f32, kind="ExternalOutput")

    n_chunks = (t_steps + CH - 1) // CH

    with tile.TileContext(nc) as tc:
        with (
            tc.tile_pool(name="const", bufs=1) as constp,
            tc.tile_pool(name="xchunk", bufs=2) as xpool,
            tc.tile_pool(name="gates", bufs=4) as gpool,
            tc.tile_pool(name="scratch", bufs=4) as spool,
            tc.tile_pool(name="ps1", bufs=3, space="PSUM") as ps1pool,
            tc.tile_pool(name="ps2", bufs=3, space="PSUM") as ps2pool,
        ):
            # --- constants / persistent state ---
            w13 = constp.tile([68, 512], f16, tag="w13")
            nc.sync.dma_start(w13[:, :], w13_d.ap()[:, :])
            w2 = constp.tile([128, 256], f16, tag="w2")
            nc.sync.dma_start(w2[:, :], w2_d.ap()[:, :])

            st = constp.tile([128, bl], f16, tag="state")  # [ht1; ht2]
            nc.vector.memset(st[:, :], 0.0)
            c1t = constp.tile([128, bl], f32, tag="c1")  # c2x layer1 (rows 0-63)
            nc.vector.memset(c1t[:, :], 0.0)
            c2t = constp.tile([128, bl], f32, tag="c2")  # c2x layer2 (rows 64-127)
            nc.vector.memset(c2t[:, :], 0.0)
            c1 = c1t[0:64, :]
            c2 = c2t[64:128, :]

            x_tiles = [None] * n_chunks

            def get_xchunk(ci):
                if x_tiles[ci] is None:
                    xt = xpool.tile([128, CH * bl], f16, tag="x")
                    lo = ci * CH * bl
                    hi = min((ci + 1) * CH, t_steps) * bl
                    nc.sync.dma_start(xt[64:68, 0 : hi - lo], xt_d.ap()[:, lo:hi])
                    x_tiles[ci] = xt
                return x_tiles[ci]

            def xslice(t):
                ci, off = divmod(t, CH)
                return get_xchunk(ci)[64:68, off * bl : (off + 1) * bl]

            # Per-layer step state handles
            ps2_of = {}  # step -> psum tile of layer-2 gates

            def l1_mms(t):
                """Layer-1 gate matmuls for step t -> psum (128, 2*bl)."""
                ps = ps1pool.tile([128, 512], f32, tag="ps1", name="ps1")[:, 0 : 2 * bl]
                xr = xslice(t)
                nc.tensor.matmul(ps[:, 0:bl], w13[64:68, 0:128], xr,
                                 start=True, stop=False)
                nc.tensor.matmul(ps[:, bl : 2 * bl], w13[64:68, 128:256], xr,
                                 start=False, stop=False)
                nc.tensor.matmul(ps[:, 0:bl], w13[0:64, 0:128], st[0:64, :],
                                 start=False, stop=False)
                nc.tensor.matmul(ps[:, bl : 2 * bl], w13[0:64, 128:256],
                                 st[0:64, :], start=False, stop=True)
                return ps

            def l2_mms(t):
                """Layer-2 gate matmuls for step t (needs ht1(t), ht2(t-1))."""
                ps = ps2pool.tile([128, 512], f32, tag="ps2", name="ps2")[:, 0 : 2 * bl]
                xr = xslice(t)  # only the ones-row matters (rows 64-66 hit zeros)
                nc.tensor.matmul(ps[:, 0:bl], w13[64:68, 256:384], xr,
                                 start=True, stop=False)
                nc.tensor.matmul(ps[:, bl : 2 * bl], w13[64:68, 384:512], xr,
                                 start=False, stop=False)
                nc.tensor.matmul(ps[:, 0:bl], w2[:, 0:128], st[:, :],
                                 start=False, stop=False)
                nc.tensor.matmul(ps[:, bl : 2 * bl], w2[:, 128:256], st[:, :],
                                 start=False, stop=True)
                ps2_of[t] = ps

            def slices_of(t1, layer):
                """Layer 1 gate col order [f,i,o,g]: algebra rows 0-63.
                Layer 2 gate col order [i,f,g,o]: algebra rows 64-127."""
                if layer == 1:
                    lo = slice(0, 64)
                    tf, ti = t1[0:64, 0:bl], t1[64:128, 0:bl]
                    to, tg = t1[0:64, bl : 2 * bl], t1[64:128, bl : 2 * bl]
                else:
                    lo = slice(64, 128)
                    ti, tf = t1[0:64, 0:bl], t1[64:128, 0:bl]
                    tg, to = t1[0:64, bl : 2 * bl], t1[64:128, bl : 2 * bl]
                return lo, ti, tf, tg, to

            def cell_a(ps, layer):
                """ACT: tanh over all four gate blocks."""
                t1 = gpool.tile([128, 2 * bl], f16, tag=f"t1l{layer}",
                                name=f"t1l{layer}")
                nc.scalar.activation(t1[:, :], ps[:, :], Tanh)
                return t1

            def cell_b(t1, cc, layer):
                """DVE cell update in 3 ops:
                u = (ti+1)*tg = 2ig;  w = (tf+1)*c2x = 4fc;
                c2x = 0.5*w + u = 2(fc + ig)."""
                lo, ti, tf, tg, to = slices_of(t1, layer)
                u = spool.tile([128, bl], f16, tag=f"u{layer}", name=f"u{layer}")[lo, :]
                nc.vector.scalar_tensor_tensor(u, ti, 1.0, tg, ADD, MULT)
                w = spool.tile([128, bl], f32, tag=f"w{layer}", name=f"w{layer}")[lo, :]
                nc.vector.scalar_tensor_tensor(w, tf, 1.0, cc, ADD, MULT)
                nc.vector.scalar_tensor_tensor(cc, w, 0.5, u, MULT, ADD)

            def cell_c(t1, cc, layer):
                """ACT tanh(c) + DVE ht = (to+1)*tc -> st."""
                lo, ti, tf, tg, to = slices_of(t1, layer)
                tcl = spool.tile([128, bl], f16, tag=f"tc{layer}",
                                 name=f"tc{layer}")[lo, :]
                nc.scalar.activation(tcl, cc, Tanh, scale=0.5)
                nc.vector.scalar_tensor_tensor(st[lo, :], to, 1.0, tcl, ADD, MULT)

            # Emission order = per-engine queue order.  Interleave the two
            # layer chains (L2 runs one step behind L1) so neither chain
            # head-of-line-blocks the other on the ACT/DVE FIFOs.
            for t in range(t_steps):
                ps1 = l1_mms(t)
                if t >= 1:
                    l2_mms(t - 1)
                t1b = cell_a(ps2_of.pop(t - 1), 2) if t >= 1 else None
                t1a = cell_a(ps1, 1)
                if t1b is not None:
                    cell_b(t1b, c2, 2)
                cell_b(t1a, c1, 1)
                if t1b is not None:
                    cell_c(t1b, c2, 2)  # writes ht2(t-1)
                cell_c(t1a, c1, 1)  # writes ht1(t)
                # free old x chunk handle (keeps python refs bounded)
                ci = t // CH
                if ci >= 2:
                    x_tiles[ci - 2] = None

            l2_mms(t_steps - 1)
            t1b = cell_a(ps2_of.pop(t_steps - 1), 2)
            cell_b(t1b, c2, 2)
            cell_c(t1b, c2, 2)

            # out = 0.5 * ht2 = h2_final (transposed: H x batch), fp32
            ob = constp.tile([128, bl], f32, tag="out")
            nc.vector.tensor_scalar_mul(ob[64:128, :], st[64:128, :], 0.5)
            nc.sync.dma_start(out_d.ap()[:, :], ob[64:128, :])

    nc.compile()
    return nc


def _get_program(t_steps=T):
    key = ("prog", t_steps)
    if key not in _CACHE:
        _CACHE[key] = build_program(t_steps)
    return _CACHE[key]


def kernel(x, W_ih0, W_hh0, b_ih0, b_hh0, W_ih1, W_hh1, b_ih1, b_hh1):
    from concourse import bass_utils

    x = np.asarray(x, np.float32)
    w13, w2 = _prep_weights(
        np.asarray(W_ih0, np.float32), np.asarray(W_hh0, np.float32),
        np.asarray(b_ih0, np.float32), np.asarray(b_hh0, np.float32),
        np.asarray(W_ih1, np.float32), np.asarray(W_hh1, np.float32),
        np.asarray(b_ih1, np.float32), np.asarray(b_hh1, np.float32),
    )

    nc = _get_program(T)

    in_maps = []
    for c in range(NCORES):
        xc = x[c * BL : (c + 1) * BL]  # (BL, T, 3)
        xt = np.ones((4, T * BL), np.float16)
        xt[0:3] = xc.transpose(2, 1, 0).reshape(3, T * BL).astype(np.float16)
        in_maps.append({"xt": xt, "w13": w13, "w2": w2})

    res = bass_utils.run_bass_kernel_spmd(nc, in_maps, core_ids=list(range(NCORES)))
    outs = [res.results[c]["out"].T for c in range(NCORES)]  # (BL, 64) each
    return np.concatenate(outs, axis=0).astype(np.float32)


if __name__ == "__main__":
    rng = np.random.default_rng(0)
    s = 1.0 / np.sqrt(H)
    inputs = {
        "x": rng.standard_normal((B, T, I), np.float32),
        "W_ih0": rng.uniform(-s, s, (4 * H, I)).astype(np.float32),
        "W_hh0": rng.uniform(-s, s, (4 * H, H)).astype(np.float32),
        "b_ih0": rng.uniform(-s, s, 4 * H).astype(np.float32),
        "b_hh0": rng.uniform(-s, s, 4 * H).astype(np.float32),
        "W_ih1": rng.uniform(-s, s, (4 * H, H)).astype(np.float32),
        "W_hh1": rng.uniform(-s, s, (4 * H, H)).astype(np.float32),
        "b_ih1": rng.uniform(-s, s, 4 * H).astype(np.float32),
        "b_hh1": rng.uniform(-s, s, 4 * H).astype(np.float32),
    }
    out = kernel(**inputs)
    print(out.shape, out.dtype, np.abs(out).max())



# revision 3
# speedup vs baseline: 1.2121x; 1.2121x over previous
"""Bass/Trainium2 kernel for a 2-layer LSTM (B=512, T=2048, I=3, H=64).

Returns the final hidden state of layer 2, shape (512, 64) fp32.

v2 strategy (data-parallel over batch, 8 cores x 64 batch each):
  Both layers advance in ONE fused chain, L2 one step behind L1.
  Layout: partition rows 0:64 = layer 1, rows 64:128 = layer 2 (step t-1).
  Gates live on the free dim as four 64-wide column blocks [i | f | g | o],
  so every elementwise cell op covers BOTH layers in a single instruction.

  Round r (r = 0..T inclusive) computes L1 step r and L2 step r-1:
    ps[128,256] (PSUM) = x/bias MMs (K=4, start) + h MMs (K=128, stop)
    t1 = tanh(ps)                  # one 256-wide ACT: all gates, both layers
    w  = (tf+1)*c2x                # = 4fc      (STT)
    u  = (ti+1)*tg                 # = 2ig      (STT)
    c2x= 0.5*w + u                 # = 2c'      (STT, in-place)
    ho = to+1                      # = 2o       (tensor_scalar, off-chain)
    tc = tanh(0.5*c2x)             # = tanh(c') (ACT)
    st = ho*tc                     # = 2h'      (tensor_tensor, in-place)

  State: st[128,64] f16 = [2*h1(r); 2*h2(r-1)], c2x[128,64] f32 = 2*c.
  sigmoid(z) = (tanh(z/2)+1)/2: 0.5 baked into i/f/o gate weights, so one
  tanh ACT covers all gates.  h stored as 2h; weights that act on it carry
  an extra 0.5.

  Round 0 uses a bias-zeroed L2 weight block so L2 state stays exactly 0
  until its real step 0 happens in round 1.
"""

import numpy as np

B, T, I, H = 512, 2048, 3, 64
NCORES = 8
BL = B // NCORES  # 64 batch per core
CH = 64  # timesteps per x-chunk DMA

_CACHE = {}

# gate scales for the tanh-sigmoid trick (i, f, g, o)
_SG = (0.5, 0.5, 1.0, 0.5)


def _prep_weights(W_ih0, W_hh0, b_ih0, b_hh0, W_ih1, W_hh1, b_ih1, b_hh1):
    """Pack host-side lhsT weight arrays (fp16).

    w4x  [4, 512]: per gate G a [4, 128] block: rows 0:3 act on x (L1 cols
         0:64), row 3 is the ones-row (biases for L1 and L2).
    w4x0 [4, 512]: round-0 variant with the L2 bias zeroed.
    wh   [128, 512]: per gate G a [128, 128] block: rows = st ([2h1; 2h2]),
         cols 0:64 -> L1 gate outs, 64:128 -> L2 gate outs.
    """
    b0 = b_ih0 + b_hh0
    b1 = b_ih1 + b_hh1
    w4x = np.zeros((4, 512), np.float32)
    wh = np.zeros((128, 512), np.float32)
    for G in range(4):
        s = _SG[G]
        gs = slice(G * 64, (G + 1) * 64)
        c0 = G * 128  # L1 col base within this gate block
        c1 = G * 128 + 64  # L2 col base
        w4x[0:3, c0 : c0 + 64] = s * W_ih0[gs, :].T
        w4x[3, c0 : c0 + 64] = s * b0[gs]
        w4x[3, c1 : c1 + 64] = s * b1[gs]
        wh[0:64, c0 : c0 + 64] = 0.5 * s * W_hh0[gs, :].T
        wh[0:64, c1 : c1 + 64] = 0.5 * s * W_ih1[gs, :].T
        wh[64:128, c1 : c1 + 64] = 0.5 * s * W_hh1[gs, :].T
    w4x0 = w4x.copy()
    w4x0[3, :] = 0.0
    for G in range(4):
        c0 = G * 128
        w4x0[3, c0 : c0 + 64] = _SG[G] * b0[G * 64 : (G + 1) * 64]
    return (
        w4x.astype(np.float16),
        w4x0.astype(np.float16),
        wh.astype(np.float16),
    )


def build_program(t_steps=T, bl=BL):
    """Build the Bass program (one core's SPMD program)."""
    import concourse.bass as bass
    import concourse.tile as tile
    from concourse import bacc, mybir

    f32 = mybir.dt.float32
    f16 = mybir.dt.float16
    Tanh = mybir.ActivationFunctionType.Tanh
    ADD = mybir.AluOpType.add
    MULT = mybir.AluOpType.mult

    nc = bacc.Bacc("TRN2", target_bir_lowering=False, debug=False)

    xt_d = nc.dram_tensor("xt", [4, t_steps * bl], f16, kind="ExternalInput")
    w4x_d = nc.dram_tensor("w4x", [4, 512], f16, kind="ExternalInput")
    w4x0_d = nc.dram_tensor("w4x0", [4, 512], f16, kind="ExternalInput")
    wh_d = nc.dram_tensor("wh", [128, 512], f16, kind="ExternalInput")
    out_d = nc.dram_tensor("out", [64, bl], f32, kind="ExternalOutput")

    n_chunks = (t_steps + CH - 1) // CH
    n_rounds = t_steps + 1

    with tile.TileContext(nc) as tc:
        with (
            tc.tile_pool(name="const", bufs=1) as constp,
            tc.tile_pool(name="xchunk", bufs=3) as xpool,
            tc.tile_pool(name="gates", bufs=3) as gpool,
            tc.tile_pool(name="scratch", bufs=2) as spool,
            tc.tile_pool(name="ps", bufs=4, space="PSUM") as pspool,
        ):
            # --- constants / persistent state ---
            w4x = constp.tile([4, 512], f16, tag="w4x")
            nc.sync.dma_start(w4x[:, :], w4x_d.ap()[:, :])
            w4x0 = constp.tile([4, 512], f16, tag="w4x0")
            nc.sync.dma_start(w4x0[:, :], w4x0_d.ap()[:, :])
            wh = constp.tile([128, 512], f16, tag="wh")
            nc.sync.dma_start(wh[:, :], wh_d.ap()[:, :])

            st = constp.tile([128, bl], f16, tag="state")  # [2h1; 2h2]
            nc.vector.memset(st[:, :], 0.0)
            c2x = constp.tile([128, bl], f32, tag="c2x")  # [2c1; 2c2]
            nc.vector.memset(c2x[:, :], 0.0)

            x_tiles = [None] * n_chunks

            def fetch_xchunk(ci):
                if ci < n_chunks and x_tiles[ci] is None:
                    xt = xpool.tile([4, CH * bl], f16, tag="x")
                    lo = ci * CH * bl
                    hi = min((ci + 1) * CH, t_steps) * bl
                    nc.sync.dma_start(xt[0:4, 0 : hi - lo], xt_d.ap()[:, lo:hi])
                    x_tiles[ci] = xt

            def xslice(t, h):
                ci, off = divmod(t, CH)
                fetch_xchunk(ci)
                fetch_xchunk(ci + 1)  # prefetch with a full chunk of lead
                base = off * bl + h * HB
                return x_tiles[ci][0:4, base : base + HB]

            HB = bl // 2  # 32 batch per chain

            def round_body(r, h, st, c2x):
                """One step for chain h (h=0: batch 0:32, h=1: 32:64)."""
                xr = xslice(min(r, t_steps - 1), h)
                wx = w4x0 if r == 0 else w4x
                psb = pspool.tile([128, 512], f32, tag=f"ps{h}", name=f"ps{h}")
                ps = psb[:, 0 : 4 * HB]
                for G in range(4):
                    nc.tensor.matmul(
                        ps[:, G * HB : (G + 1) * HB],
                        wx[0:4, G * 128 : (G + 1) * 128],
                        xr,
                        start=(G == 0),
                        stop=False,
                    )
                for G in range(4):
                    nc.tensor.matmul(
                        ps[:, G * HB : (G + 1) * HB],
                        wh[:, G * 128 : (G + 1) * 128],
                        st[:, :],
                        start=False,
                        stop=(G == 3),
                    )

                t1 = gpool.tile([128, 4 * HB], f16, tag=f"t1{h}", name=f"t1{h}")
                nc.scalar.activation(t1[:, :], ps[:, :], Tanh)
                ti = t1[:, 0:HB]
                tf = t1[:, HB : 2 * HB]
                tg = t1[:, 2 * HB : 3 * HB]
                to = t1[:, 3 * HB : 4 * HB]

                w = spool.tile([128, HB], f32, tag=f"w{h}", name=f"w{h}")
                nc.vector.scalar_tensor_tensor(w[:, :], tf, 1.0, c2x[:, :], ADD, MULT)
                u = spool.tile([128, HB], f16, tag=f"u{h}", name=f"u{h}")
                nc.vector.scalar_tensor_tensor(u[:, :], ti, 1.0, tg, ADD, MULT)
                nc.vector.scalar_tensor_tensor(c2x[:, :], w[:, :], 0.5, u[:, :], MULT, ADD)
                tcl = spool.tile([128, HB], f16, tag=f"tc{h}", name=f"tc{h}")
                nc.scalar.activation(tcl[:, :], c2x[:, :], Tanh, scale=0.5)
                ho = spool.tile([128, HB], f16, tag=f"ho{h}", name=f"ho{h}")
                nc.vector.tensor_scalar_add(ho[:, :], to, 1.0)
                nc.vector.tensor_tensor(st[:, :], ho[:, :], tcl[:, :], MULT)

            sts = [st[:, 0:HB], st[:, HB:bl]]
            c2xs = [c2x[:, 0:HB], c2x[:, HB:bl]]
            for r in range(n_rounds):
                round_body(r, 0, sts[0], c2xs[0])
                round_body(r, 1, sts[1], c2xs[1])
                ci = r // CH
                if ci >= 2:
                    x_tiles[ci - 2] = None

            # out = 0.5 * st[64:128] = h2(T-1), fp32 (H x batch, transposed)
            ob = constp.tile([128, bl], f32, tag="out")
            nc.vector.tensor_scalar_mul(ob[64:128, :], st[64:128, :], 0.5)
            nc.sync.dma_start(out_d.ap()[:, :], ob[64:128, :])

    nc.compile()
    return nc


def _get_program(t_steps=T):
    key = ("prog", t_steps)
    if key not in _CACHE:
        _CACHE[key] = build_program(t_steps)
    return _CACHE[key]


def make_in_maps(inputs, tt=T):
    """Per-core input maps for the program built with t_steps=tt."""
    x = np.asarray(inputs["x"], np.float32)
    w4x, w4x0, wh = _prep_weights(
        *(np.asarray(inputs[k], np.float32) for k in
          ("W_ih0", "W_hh0", "b_ih0", "b_hh0", "W_ih1", "W_hh1", "b_ih1", "b_hh1"))
    )
    in_maps = []
    for c in range(NCORES):
        xc = x[c * BL : (c + 1) * BL, :tt]  # (BL, tt, 3)
        xt = np.ones((4, tt * BL), np.float16)
        xt[0:3] = xc.transpose(2, 1, 0).reshape(3, tt * BL).astype(np.float16)
        in_maps.append({"xt": xt, "w4x": w4x, "w4x0": w4x0, "wh": wh})
    return in_maps


def kernel(x, W_ih0, W_hh0, b_ih0, b_hh0, W_ih1, W_hh1, b_ih1, b_hh1):
    from concourse import bass_utils

    nc = _get_program(T)
    in_maps = make_in_maps(dict(
        x=x, W_ih0=W_ih0, W_hh0=W_hh0, b_ih0=b_ih0, b_hh0=b_hh0,
        W_ih1=W_ih1, W_hh1=W_hh1, b_ih1=b_ih1, b_hh1=b_hh1))

    res = bass_utils.run_bass_kernel_spmd(nc, in_maps, core_ids=list(range(NCORES)))
    outs = [res.results[c]["out"].T for c in range(NCORES)]  # (BL, 64) each
    return np.concatenate(outs, axis=0).astype(np.float32)


if __name__ == "__main__":
    rng = np.random.default_rng(0)
    s = 1.0 / np.sqrt(H)
    inputs = {
        "x": rng.standard_normal((B, T, I), np.float32),
        "W_ih0": rng.uniform(-s, s, (4 * H, I)).astype(np.float32),
        "W_hh0": rng.uniform(-s, s, (4 * H, H)).astype(np.float32),
        "b_ih0": rng.uniform(-s, s, 4 * H).astype(np.float32),
        "b_hh0": rng.uniform(-s, s, 4 * H).astype(np.float32),
        "W_ih1": rng.uniform(-s, s, (4 * H, H)).astype(np.float32),
        "W_hh1": rng.uniform(-s, s, (4 * H, H)).astype(np.float32),
        "b_ih1": rng.uniform(-s, s, 4 * H).astype(np.float32),
        "b_hh1": rng.uniform(-s, s, 4 * H).astype(np.float32),
    }
    out = kernel(**inputs)
    print(out.shape, out.dtype, np.abs(out).max())


# revision 4
# speedup vs baseline: 1.2905x; 1.0647x over previous
"""Bass/Trainium2 kernel for a 2-layer LSTM (B=512, T=2048, I=3, H=64).

Returns the final hidden state of layer 2, shape (512, 64) fp32.

v5: raw Bass, two independent 32-batch chains per core (see kernel_v4), with
the cell DVE work collapsed to THREE ops via a layout trick: per chain one
f32 tensor C[128, 160] = [to | ti | tf | tg | c2x] (32 cols each), gates
written by ONE tanh ACT from PSUM blocks ordered [o|i|f|g].  Then:

  uw = ([ti|tf] + 1) * [tg|c2x]   -> [u|w] = [2ig | 4fc]   (one 64-wide STT)
  c2x' = 0.5*w + u                -> written back into C's c2x column
  tc   = tanh(0.5*c2x')           (ACT)
  st   = (to + 1) * tc            -> [2h1; 2h2]            (STT, f16 out)

Chain per round: st-sem -> 4 h-MMs -> gates-ACT -> uw -> c -> tc-ACT -> st.
c2x stays f32 (f16 cell state loses ~0.5 rel err over T=2048).
"""

import numpy as np

B, T, I, H = 512, 2048, 3, 64
NCORES = 8
BL = B // NCORES  # 64 batch per core
HB = BL // 2  # 32 batch per chain
CH = 64  # timesteps per x-chunk DMA
NXR = 3  # x-chunk ring buffers

_CACHE = {}

# PSUM/C gate block order is [o, i, f, g]; PyTorch weight rows are [i, f, g, o]
_BLK = ((3, 0.5), (0, 0.5), (1, 0.5), (2, 1.0))  # (row-block, tanh-trick scale)


def _prep_weights(W_ih0, W_hh0, b_ih0, b_hh0, W_ih1, W_hh1, b_ih1, b_hh1):
    b0 = b_ih0 + b_hh0
    b1 = b_ih1 + b_hh1
    w4x = np.zeros((4, 512), np.float32)
    wh = np.zeros((128, 512), np.float32)
    for G, (rb, s) in enumerate(_BLK):
        gs = slice(rb * 64, (rb + 1) * 64)
        c0 = G * 128
        c1 = G * 128 + 64
        w4x[0:3, c0 : c0 + 64] = s * W_ih0[gs, :].T
        w4x[3, c0 : c0 + 64] = s * b0[gs]
        w4x[3, c1 : c1 + 64] = s * b1[gs]
        wh[0:64, c0 : c0 + 64] = 0.5 * s * W_hh0[gs, :].T
        wh[0:64, c1 : c1 + 64] = 0.5 * s * W_ih1[gs, :].T
        wh[64:128, c1 : c1 + 64] = 0.5 * s * W_hh1[gs, :].T
    w4x0 = w4x.copy()
    w4x0[3, :] = 0.0
    for G, (rb, s) in enumerate(_BLK):
        w4x0[3, G * 128 : G * 128 + 64] = s * b0[rb * 64 : (rb + 1) * 64]
    return (
        w4x.astype(np.float16),
        w4x0.astype(np.float16),
        wh.astype(np.float16),
    )


def build_program(t_steps=T):
    import concourse.bass as bass
    from concourse import bacc, mybir

    f32 = mybir.dt.float32
    f16 = mybir.dt.float16
    Tanh = mybir.ActivationFunctionType.Tanh
    ADD = mybir.AluOpType.add
    MULT = mybir.AluOpType.mult

    nc = bacc.Bacc("TRN2", target_bir_lowering=False, debug=False)

    xt_d = nc.dram_tensor("xt", [4, t_steps * BL], f16, kind="ExternalInput")
    w4x_d = nc.dram_tensor("w4x", [4, 512], f16, kind="ExternalInput")
    w4x0_d = nc.dram_tensor("w4x0", [4, 512], f16, kind="ExternalInput")
    wh_d = nc.dram_tensor("wh", [128, 512], f16, kind="ExternalInput")
    out_d = nc.dram_tensor("out", [64, BL], f32, kind="ExternalOutput")

    n_chunks = (t_steps + CH - 1) // CH
    n_rounds = t_steps + 1

    w4x = nc.alloc_sbuf_tensor("w4xs", [4, 512], f16)
    w4x0 = nc.alloc_sbuf_tensor("w4x0s", [4, 512], f16)
    wh = nc.alloc_sbuf_tensor("whs", [128, 512], f16)
    st = nc.alloc_sbuf_tensor("sts", [128, BL], f16)
    xb = nc.alloc_sbuf_tensor("xbs", [4, NXR * CH * BL], f16)
    ob = nc.alloc_sbuf_tensor("obs", [128, BL], f32)
    C = [nc.alloc_sbuf_tensor(f"cc{x}", [128, 5 * HB], f32) for x in "ab"]
    UW = [nc.alloc_sbuf_tensor(f"uw{x}", [128, 2 * HB], f32) for x in "ab"]
    TC = [nc.alloc_sbuf_tensor(f"tc{x}", [128, HB], f16) for x in "ab"]
    ps = [nc.alloc_psum_tensor(f"ps{x}", [128, 3 * 512], f32) for x in "ab"]

    sem_w = nc.alloc_semaphore("sem_w")
    sem_x = nc.alloc_semaphore("sem_x")
    sem_out = nc.alloc_semaphore("sem_out")
    sem_done = nc.alloc_semaphore("sem_done")
    sem_mm = [nc.alloc_semaphore(f"sem_mm{x}") for x in "ab"]
    sem_mmo = [nc.alloc_semaphore(f"sem_mmo{x}") for x in "ab"]
    sem_o = [nc.alloc_semaphore(f"sem_o{x}") for x in "ab"]
    sem_act = [nc.alloc_semaphore(f"sem_act{x}") for x in "ab"]
    sem_uw = [nc.alloc_semaphore(f"sem_uw{x}") for x in "ab"]
    sem_c = [nc.alloc_semaphore(f"sem_c{x}") for x in "ab"]
    sem_tc = [nc.alloc_semaphore(f"sem_tc{x}") for x in "ab"]
    sem_st = [nc.alloc_semaphore(f"sem_st{x}") for x in "ab"]

    stv = [st[0:128, 0:HB], st[0:128, HB:BL]]

    def xsl(r, h):
        t = min(r, t_steps - 1)
        ci, off = divmod(t, CH)
        base = (ci % NXR) * CH * BL + off * BL + h * HB
        return xb[0:4, base : base + HB]

    def psv(h, r, lo, hi):
        base = (r % 3) * 512
        return ps[h][0:128, base + lo : base + hi]

    with nc.Block() as blk:

        @blk.sync
        def _(sync: bass.BassEngine):
            sync.dma_start(w4x[:, :], w4x_d.ap()[:, :]).then_inc(sem_w, 16)
            sync.dma_start(w4x0[:, :], w4x0_d.ap()[:, :]).then_inc(sem_w, 16)
            sync.dma_start(wh[:, :], wh_d.ap()[:, :]).then_inc(sem_w, 16)
            for ci in range(n_chunks):
                lo = ci * CH * BL
                hi = min((ci + 1) * CH, t_steps) * BL
                base = (ci % NXR) * CH * BL
                d = sync.dma_start(
                    xb[0:4, base : base + hi - lo], xt_d.ap()[:, lo:hi]
                ).then_inc(sem_x, 16)
                if ci >= NXR:
                    d._wait_ge(sem_st[1], (ci - NXR + 1) * CH + 1)
            sync.wait_ge(sem_out, 1)
            sync.dma_start(out_d.ap()[:, :], ob[64:128, 0:BL]).then_inc(sem_done, 16)
            sync.wait_ge(sem_done, 16)

        @blk.tensor
        def _(pe: bass.BassEngine):
            pe.wait_ge(sem_w, 48)
            for r in range(n_rounds):
                wx = w4x0 if r == 0 else w4x
                if r % CH == 0:
                    ci = min(r, t_steps - 1) // CH
                    pe.wait_ge(sem_x, (ci + 1) * 16)
                for h in range(2):
                    xr = xsl(r, h)
                    for G in range(4):
                        mm = pe.matmul(
                            psv(h, r, G * HB, (G + 1) * HB),
                            wx[0:4, G * 128 : (G + 1) * 128],
                            xr,
                            start=(G == 0),
                            stop=False,
                        )
                        if G == 0 and r >= 3:
                            mm._wait_ge(sem_act[h], r - 2)
                    for G in range(4):
                        mm = pe.matmul(
                            psv(h, r, G * HB, (G + 1) * HB),
                            wh[0:128, G * 128 : (G + 1) * 128],
                            stv[h],
                            start=False,
                            stop=(G == 3),
                        )
                        if G == 0:
                            mm._wait_ge(sem_st[h], r + 1)
                        if G == 3:
                            mm.then_inc(sem_mm[h], 1)

        @blk.scalar
        def _(act: bass.BassEngine):
            for r in range(n_rounds):
                for h in range(2):
                    act.activation(
                        C[h][0:128, 0 : 4 * HB], psv(h, r, 0, 4 * HB), Tanh
                    )._wait_ge(sem_mm[h], r + 1).then_inc(sem_act[h], 1)
                for h in range(2):
                    act.activation(
                        TC[h][0:128, 0:HB], C[h][0:128, 4 * HB : 5 * HB],
                        Tanh, scale=0.5,
                    )._wait_ge(sem_c[h], r + 2).then_inc(sem_tc[h], 1)

        @blk.vector
        def _(v: bass.BassEngine):
            for h in range(2):
                v.memset(C[h][0:128, 4 * HB : 5 * HB], 0.0).then_inc(sem_c[h], 1)
            v.memset(stv[0], 0.0).then_inc(sem_st[0], 1)
            v.memset(stv[1], 0.0).then_inc(sem_st[1], 1)
            v.wait_ge(sem_c[0], 1)
            v.wait_ge(sem_c[1], 1)
            for r in range(n_rounds):
                for h in range(2):
                    v.scalar_tensor_tensor(
                        UW[h][0:128, 0 : 2 * HB],
                        C[h][0:128, HB : 3 * HB],
                        1.0,
                        C[h][0:128, 3 * HB : 5 * HB],
                        ADD,
                        MULT,
                    )._wait_ge(sem_act[h], r + 1).then_inc(sem_uw[h], 1)
                    v.scalar_tensor_tensor(
                        C[h][0:128, 4 * HB : 5 * HB],
                        UW[h][0:128, HB : 2 * HB],
                        0.5,
                        UW[h][0:128, 0:HB],
                        MULT,
                        ADD,
                    )._wait_ge(sem_uw[h], r + 1).then_inc(sem_c[h], 1)
                for h in range(2):
                    v.scalar_tensor_tensor(
                        stv[h], C[h][0:128, 0:HB], 1.0, TC[h][0:128, 0:HB],
                        ADD, MULT,
                    )._wait_ge(sem_tc[h], r + 1).then_inc(sem_st[h], 1)
            v.wait_ge(sem_st[0], n_rounds + 1)
            v.tensor_scalar_mul(ob[64:128, 0:BL], st[64:128, 0:BL], 0.5)._wait_ge(
                sem_st[1], n_rounds + 1
            ).then_inc(sem_out, 1)

    nc.compile()
    return nc


def _get_program(t_steps=T):
    key = ("prog", t_steps)
    if key not in _CACHE:
        _CACHE[key] = build_program(t_steps)
    return _CACHE[key]


def make_in_maps(inputs, tt=T):
    x = np.asarray(inputs["x"], np.float32)
    w4x, w4x0, wh = _prep_weights(
        *(np.asarray(inputs[k], np.float32) for k in
          ("W_ih0", "W_hh0", "b_ih0", "b_hh0", "W_ih1", "W_hh1", "b_ih1", "b_hh1"))
    )
    in_maps = []
    for c in range(NCORES):
        xc = x[c * BL : (c + 1) * BL, :tt]
        xt = np.ones((4, tt * BL), np.float16)
        xt[0:3] = xc.transpose(2, 1, 0).reshape(3, tt * BL).astype(np.float16)
        in_maps.append({"xt": xt, "w4x": w4x, "w4x0": w4x0, "wh": wh})
    return in_maps


def kernel(x, W_ih0, W_hh0, b_ih0, b_hh0, W_ih1, W_hh1, b_ih1, b_hh1):
    from concourse import bass_utils

    nc = _get_program(T)
    in_maps = make_in_maps(dict(
        x=x, W_ih0=W_ih0, W_hh0=W_hh0, b_ih0=b_ih0, b_hh0=b_hh0,
        W_ih1=W_ih1, W_hh1=W_hh1, b_ih1=b_ih1, b_hh1=b_hh1))

    res = bass_utils.run_bass_kernel_spmd(nc, in_maps, core_ids=list(range(NCORES)))
    outs = [res.results[c]["out"].T for c in range(NCORES)]
    return np.concatenate(outs, axis=0).astype(np.float32)


if __name__ == "__main__":
    rng = np.random.default_rng(0)
    s = 1.0 / np.sqrt(H)
    inputs = {
        "x": rng.standard_normal((B, T, I), np.float32),
        "W_ih0": rng.uniform(-s, s, (4 * H, I)).astype(np.float32),
        "W_hh0": rng.uniform(-s, s, (4 * H, H)).astype(np.float32),
        "b_ih0": rng.uniform(-s, s, 4 * H).astype(np.float32),
        "b_hh0": rng.uniform(-s, s, 4 * H).astype(np.float32),
        "W_ih1": rng.uniform(-s, s, (4 * H, H)).astype(np.float32),
        "W_hh1": rng.uniform(-s, s, (4 * H, H)).astype(np.float32),
        "b_ih1": rng.uniform(-s, s, 4 * H).astype(np.float32),
        "b_hh1": rng.uniform(-s, s, 4 * H).astype(np.float32),
    }
    out = kernel(**inputs)
    print(out.shape, out.dtype, np.abs(out).max())
